# revision 7
# baseline (speedup 1.0000x reference)
"""Trainium2 Bass kernel for nn_AttnGate (per-k-head gated attention scores).

Computes, for each k-head h:
  qp   = einsum('sgi,gio->so', q[:, 4h:4h+4, :], wq[h])        (T, DG)
  qn   = rope(rmsnorm(qp)) per padded position                 (per batch)
  kk   = rope(rmsnorm([blockmax(k); blockavg(k)] @ wk[h]))     (nb, DG)
  out  = where(mask, qn @ kk.T / sqrt(DG), -1e20)              (S, nb)

Sharding: one k-head per NeuronCore (8 heads, 8 cores). All device work in
transposed layout (feature on partitions, sequence on free dim):
  - Q projection:  psum(DG, s) = wq_chunk(K=128, DG).T @ qT_chunk(K=128, s)
  - RMSNorm across partitions via ones-column matmul (sumsq), Dsqrt
    activation (0.5/sqrt(x)), K=1 broadcast matmul.  Norm scale factors
    fold into the broadcast lhsT consts (2.0 q-side incl 1/sqrt(DG),
    2*sqrt(128) k-side).
  - RoPE as elementwise ops with partition-shifted slices; sin tables are
    pre-negated on the host for the low half.
  - Scores: psum(nbv, s) = zk(K=DG, nbv).T @ wn(K=DG, s); additive causal
    mask (-1e20) applied with the psum->sbuf move.

The host does only sharding glue: per-head transposes, table prep, and the
final gather/transpose plus -1e20 fill of padded rows/blocks.
"""

import math
from dataclasses import dataclass

import numpy as np

import concourse.bacc as bacc
import concourse.bass as bass
import concourse.mybir as mybir
from concourse import tile
from concourse.bass_utils import run_bass_kernel_spmd

BLOCK = 64
KH = 8
G = 4
D = 128
DG = 128
EPS = 1e-6
N_CORES = 8
STILE = 512  # s-tile width (free dim) for the q pipeline

F32 = mybir.dt.float32
F32R = mybir.dt.float32r
BF16 = mybir.dt.bfloat16


@dataclass(frozen=True)
class KCfg:
    mm_dt: str = "f32r"   # q-projection matmul dtype: f32 | f32r
    qk_dt: str = "f32r"   # QK score matmul dtype:     f32 | f32r | bf16
    aux_dt: str = "f32r"  # sumsq/broadcast matmuls:   f32 | f32r
    rope_bf16: bool = False  # rope stage in bf16 (ACT converts psum->bf16)
    loop_n: int = 1       # repeat body in a hardware loop (timing only)


def _mm_ap(ap, dt_name):
    if dt_name == "f32":
        return ap
    if dt_name == "f32r":
        return ap.bitcast(F32R)
    raise ValueError(dt_name)


def build_program(lens, cfg: KCfg = KCfg()):
    """Build + compile the per-core (single-head) SPMD program.

    lens: per-batch valid lengths, each divisible by BLOCK. The q/k inputs
    are packed (no padding): batch b occupies columns [cu[b], cu[b+1]).
    """
    lens = [int(x) for x in lens]
    assert all(L % BLOCK == 0 and L > 0 for L in lens)
    cu = np.concatenate([[0], np.cumsum(lens)]).astype(int)
    ttot = int(cu[-1])
    maxs = max(lens)
    nbmax = maxs // BLOCK

    nc = bacc.Bacc(None, target_bir_lowering=False, debug=False)

    qT = nc.dram_tensor("qT", [G * D, ttot], F32, kind="ExternalInput").ap()
    kT = nc.dram_tensor("kT", [D, ttot], F32, kind="ExternalInput").ap()
    wq = nc.dram_tensor("wq", [G * D, DG], F32, kind="ExternalInput").ap()
    wk = nc.dram_tensor("wk", [2 * D, DG], F32, kind="ExternalInput").ap()
    cosq = nc.dram_tensor("cosq", [DG, maxs], F32, kind="ExternalInput").ap()
    sinq = nc.dram_tensor("sinq", [DG, maxs], F32, kind="ExternalInput").ap()
    cosk = nc.dram_tensor("cosk", [DG, nbmax], F32, kind="ExternalInput").ap()
    sink = nc.dram_tensor("sink", [DG, nbmax], F32, kind="ExternalInput").ap()
    maskadd = nc.dram_tensor("maskadd", [BLOCK, maxs], F32, kind="ExternalInput").ap()
    outT = nc.dram_tensor("outT", [BLOCK, ttot], F32, kind="ExternalOutput").ap()

    rope_dt = BF16 if cfg.rope_bf16 else F32
    # the QK matmul consumes the rope output, so its dtype follows rope_bf16
    qk_dt = "bf16" if cfg.rope_bf16 else (
        "f32r" if cfg.qk_dt == "bf16" else cfg.qk_dt)
    nq = G * D // 128  # 4 contraction chunks for the q projection

    with tile.TileContext(nc) as tc:
        with (
            tc.tile_pool(name="consts", bufs=1) as cpool,
            tc.tile_pool(name="kin", bufs=2) as kpool,
            tc.tile_pool(name="kside", bufs=2) as zpool,
            tc.tile_pool(name="qin", bufs=3) as qpool,
            tc.tile_pool(name="rope", bufs=2) as rpool,
            tc.tile_pool(name="outp", bufs=4) as opool,
            tc.tile_pool(name="pP", bufs=2, space="PSUM") as pP,
            tc.tile_pool(name="pB", bufs=2, space="PSUM") as pB,
            tc.tile_pool(name="pS", bufs=2, space="PSUM") as pS,
            tc.tile_pool(name="pSC", bufs=2, space="PSUM") as pSC,
        ):
            # ---- resident constants ----
            wq_sb = cpool.tile([128, nq, 128], F32, name="wq_sb")
            nc.sync.dma_start(wq_sb[:], wq.rearrange("(c p) d -> p c d", p=128))
            wk_sb = cpool.tile([128, 2, 128], F32, name="wk_sb")
            nc.sync.dma_start(wk_sb[:], wk.rearrange("(c p) d -> p c d", p=128))
            cosq_sb = cpool.tile([DG, maxs], rope_dt, name="cosq_sb")
            sinq_sb = cpool.tile([DG, maxs], rope_dt, name="sinq_sb")
            cosk_sb = cpool.tile([DG, nbmax], F32, name="cosk_sb")
            sink_sb = cpool.tile([DG, nbmax], F32, name="sink_sb")
            if cfg.rope_bf16:
                # stage f32 tables, convert once via ACT
                cq32 = cpool.tile([DG, maxs], F32, name="cq32")
                sq32 = cpool.tile([DG, maxs], F32, name="sq32")
                nc.sync.dma_start(cq32[:], cosq[:])
                nc.sync.dma_start(sq32[:], sinq[:])
                nc.scalar.copy(cosq_sb[:], cq32[:])
                nc.scalar.copy(sinq_sb[:], sq32[:])
            else:
                nc.sync.dma_start(cosq_sb[:], cosq[:])
                nc.sync.dma_start(sinq_sb[:], sinq[:])
            nc.sync.dma_start(cosk_sb[:], cosk[:])
            nc.sync.dma_start(sink_sb[:], sink[:])
            mask_sb = cpool.tile([BLOCK, maxs], F32, name="mask_sb")
            nc.sync.dma_start(mask_sb[:], maskadd[:])

            ones_col = cpool.tile([128, 1], F32, name="ones_col")
            nc.vector.memset(ones_col[:], 1.0)
            # rsq = Exp(-0.5*Ln(ss + 128eps)) = (ss + 128eps)^-1/2
            # q-side bcast row folds 1/sqrt(DG): 1/sqrt(ms+eps)/sqrt(128)
            qrow = cpool.tile([1, 128], F32, name="qrow")
            nc.vector.memset(qrow[:], 1.0)
            # k-side bcast row: sqrt(128)*rsq == 1/sqrt(ms+eps)
            krow = cpool.tile([1, 128], F32, name="krow")
            nc.vector.memset(krow[:], math.sqrt(128.0))
            epsb = cpool.tile([128, 1], F32, name="epsb")
            nc.vector.memset(epsb[:], float(128.0 * EPS))

            def body():
                for b, Lb in enumerate(lens):
                    nbv = Lb // BLOCK
                    c0 = int(cu[b])
                    # ---------- K path ----------
                    kt = kpool.tile([128, maxs], F32, name="kt", tag="kt")
                    nc.sync.dma_start(kt[:, :Lb], kT[:, c0 : c0 + Lb])
                    kview = kt[:, :Lb].rearrange("p (n c) -> p n c", c=BLOCK)
                    kmax = zpool.tile([128, nbmax], F32, name="kmax", tag="kmax")
                    nc.vector.reduce_max(kmax[:, :nbv], kview, axis=mybir.AxisListType.X)
                    ksum = zpool.tile([128, nbmax], F32, name="ksum", tag="ksum")
                    nc.vector.reduce_sum(ksum[:, :nbv], kview, axis=mybir.AxisListType.X)
                    kkp = pP.tile([128, STILE], F32, name="kkp", tag="P")[:, :nbv]
                    # wk low half is pre-scaled by 1/BLOCK on the host
                    nc.tensor.matmul(kkp, _mm_ap(wk_sb[:, 0, :], cfg.aux_dt),
                                     _mm_ap(kmax[:, :nbv], cfg.aux_dt),
                                     start=True, stop=False)
                    nc.tensor.matmul(kkp, _mm_ap(wk_sb[:, 1, :], cfg.aux_dt),
                                     _mm_ap(ksum[:, :nbv], cfg.aux_dt),
                                     start=False, stop=True)
                    sqk = zpool.tile([128, nbmax], F32, name="sqk", tag="sqk")
                    nc.scalar.square(sqk[:, :nbv], kkp)
                    ssk = pS.tile([1, STILE], F32, name="ssk", tag="SS")[:, :nbv]
                    nc.tensor.matmul(ssk, _mm_ap(ones_col[:], cfg.aux_dt),
                                     _mm_ap(sqk[:, :nbv], cfg.aux_dt),
                                     start=True, stop=True)
                    lnk = zpool.tile([1, nbmax], F32, name="lnk", tag="lnk")
                    nc.scalar.activation(lnk[:, :nbv], ssk,
                                         mybir.ActivationFunctionType.Ln,
                                         bias=epsb[:1, :])
                    rsqk = zpool.tile([1, nbmax], F32, name="rsqk", tag="rsqk")
                    nc.scalar.activation(rsqk[:, :nbv], lnk[:, :nbv],
                                         mybir.ActivationFunctionType.Exp,
                                         scale=-0.5)
                    bk = pB.tile([128, STILE], F32, name="bk", tag="B")[:, :nbv]
                    nc.tensor.matmul(bk, _mm_ap(krow[:], cfg.aux_dt),
                                     _mm_ap(rsqk[:, :nbv], cfg.aux_dt),
                                     start=True, stop=True)
                    # rope(kk) * bk -> zk
                    t1k = zpool.tile([128, nbmax], F32, name="t1k", tag="t1k")
                    nc.vector.tensor_mul(t1k[:, :nbv], kkp, cosk_sb[:, :nbv])
                    t2k = zpool.tile([128, nbmax], F32, name="t2k", tag="t2k")
                    nc.vector.tensor_mul(t2k[:64, :nbv], kkp[64:128, :], sink_sb[:64, :nbv])
                    nc.vector.tensor_mul(t2k[64:128, :nbv], kkp[:64, :], sink_sb[64:128, :nbv])
                    wkk = zpool.tile([128, nbmax], F32, name="wkk", tag="wkk")
                    nc.vector.tensor_add(wkk[:, :nbv], t1k[:, :nbv], t2k[:, :nbv])
                    zk = zpool.tile([128, nbmax],
                                    BF16 if qk_dt == "bf16" else F32,
                                    name="zk", tag="zk")
                    nc.vector.tensor_mul(zk[:, :nbv], wkk[:, :nbv], bk)

                    # ---------- Q path ----------
                    qT3 = qT.rearrange("(c p) t -> p c t", p=128)
                    n_tiles = (Lb + STILE - 1) // STILE
                    for j in range(n_tiles):
                        s0 = j * STILE
                        w = min(STILE, Lb - s0)
                        qt = qpool.tile([128, nq, STILE], F32, name="qt", tag="qt")
                        nc.sync.dma_start(qt[:, :, :w], qT3[:, :, c0 + s0 : c0 + s0 + w])
                        P = pP.tile([128, STILE], F32, name="P", tag="P")[:, :w]
                        for c in range(nq):
                            nc.tensor.matmul(P, _mm_ap(wq_sb[:, c, :], cfg.mm_dt),
                                             _mm_ap(qt[:, c, :w], cfg.mm_dt),
                                             start=(c == 0), stop=(c == nq - 1))
                        sq = rpool.tile([128, STILE], F32, name="sq", tag="sq")
                        nc.scalar.square(sq[:, :w], P)
                        ss = pS.tile([1, STILE], F32, name="ss", tag="SS")[:, :w]
                        nc.tensor.matmul(ss, _mm_ap(ones_col[:], cfg.aux_dt),
                                         _mm_ap(sq[:, :w], cfg.aux_dt),
                                         start=True, stop=True)
                        lnq = rpool.tile([1, STILE], F32, name="lnq", tag="lnq")
                        nc.scalar.activation(lnq[:, :w], ss,
                                             mybir.ActivationFunctionType.Ln,
                                             bias=epsb[:1, :])
                        rsq = rpool.tile([1, STILE], F32, name="rsq", tag="rsq")
                        nc.scalar.activation(rsq[:, :w], lnq[:, :w],
                                             mybir.ActivationFunctionType.Exp,
                                             scale=-0.5)
                        B = pB.tile([128, STILE], F32, name="B", tag="B")[:, :w]
                        nc.tensor.matmul(B, _mm_ap(qrow[:], cfg.aux_dt),
                                         _mm_ap(rsq[:, :w], cfg.aux_dt),
                                         start=True, stop=True)
                        if cfg.rope_bf16:
                            y = rpool.tile([128, STILE], BF16, name="y", tag="y")
                            nc.scalar.copy(y[:, :w], P)
                            Bs = rpool.tile([128, STILE], BF16, name="Bs", tag="Bs")
                            nc.scalar.copy(Bs[:, :w], B)
                        else:
                            y, Bs = P, B
                        yw = y[:, :w] if cfg.rope_bf16 else P
                        t1 = rpool.tile([128, STILE], rope_dt, name="t1", tag="t1")
                        nc.vector.tensor_mul(t1[:, :w], yw, cosq_sb[:, s0 : s0 + w])
                        t2 = rpool.tile([128, STILE], rope_dt, name="t2", tag="t2")
                        nc.vector.tensor_mul(t2[:64, :w], yw[64:128, :] if cfg.rope_bf16 else P[64:128, :],
                                             sinq_sb[:64, s0 : s0 + w])
                        nc.vector.tensor_mul(t2[64:128, :w], yw[:64, :] if cfg.rope_bf16 else P[:64, :],
                                             sinq_sb[64:128, s0 : s0 + w])
                        wr = rpool.tile([128, STILE], rope_dt, name="wr", tag="wr")
                        nc.vector.tensor_add(wr[:, :w], t1[:, :w], t2[:, :w])
                        wn = rpool.tile([128, STILE], rope_dt, name="wn", tag="wn")
                        nc.vector.tensor_mul(wn[:, :w], wr[:, :w],
                                             Bs[:, :w] if cfg.rope_bf16 else B)
                        S = pSC.tile([BLOCK, STILE], F32, name="S", tag="SC")[:nbv, :w]
                        if qk_dt == "bf16":
                            nc.tensor.matmul(S, zk[:, :nbv], wn[:, :w],
                                             start=True, stop=True)
                        else:
                            nc.tensor.matmul(S, _mm_ap(zk[:, :nbv], qk_dt),
                                             _mm_ap(wn[:, :w], qk_dt),
                                             start=True, stop=True)
                        osb = opool.tile([BLOCK, STILE], F32, name="osb", tag="osb")
                        nc.vector.tensor_add(osb[:nbv, :w], S, mask_sb[:nbv, s0 : s0 + w])
                        nc.sync.dma_start(outT[0:nbv, c0 + s0 : c0 + s0 + w], osb[:nbv, :w])

            if cfg.loop_n > 1:
                with tc.For_i(0, cfg.loop_n, 1):
                    body()
            else:
                body()

    nc.compile()
    return nc


# ---------------------------------------------------------------------------
# Host-side orchestration
# ---------------------------------------------------------------------------

_PROG_CACHE: dict = {}


def _get_program(lens, cfg: KCfg):
    key = (tuple(lens), cfg)
    if key not in _PROG_CACHE:
        _PROG_CACHE[key] = build_program(lens, cfg)
    return _PROG_CACHE[key]


def _host_prep(q, k, wq, wk, qnorm_w, knorm_w, cos_q, sin_q, cos_k, sin_k,
               lens, maxs):
    """Build the 8 per-core input dicts (fast path, packed layout)."""
    T = q.shape[0]
    nbmax = maxs // BLOCK
    lens = [int(x) for x in lens]

    # rope tables (identical across batches on the fast path)
    cosqT = np.ascontiguousarray(cos_q[0].T)          # (DG, maxs)
    sinqT = np.ascontiguousarray(sin_q[0].T)
    sinqT_adj = sinqT.copy()
    sinqT_adj[: DG // 2] *= -1.0
    coskT = np.ascontiguousarray(cos_k[0].T)          # (DG, nbmax)
    sinkT_adj = np.ascontiguousarray(sin_k[0].T)
    sinkT_adj = sinkT_adj.copy()
    sinkT_adj[: DG // 2] *= -1.0

    # additive causal mask in (t, s_local) layout
    s_loc = np.arange(maxs)
    t_idx = np.arange(BLOCK)
    mask_add = np.where((s_loc[None, :] // BLOCK) >= t_idx[:, None],
                        0.0, -1e20).astype(np.float32)

    # rmsnorm weights are folded in only if they are all-ones (reference
    # always uses ones); otherwise fold into wq/wk? they multiply post-norm,
    # positionwise over DG — fold into cos/sin tables instead.
    qw = np.asarray(qnorm_w, np.float32)
    kw = np.asarray(knorm_w, np.float32)
    if not np.allclose(qw, 1.0):
        cosqT = cosqT * qw[:, None]
        rot_w = np.concatenate([qw[DG // 2:], qw[: DG // 2]])
        # rope mixes dims: y*w -> (y*w)*cos + rot(y*w)*sin. w applies to y
        # BEFORE rope per reference.  rot(y*w)[d] uses w[rot(d)].
        sinqT_adj = sinqT_adj * rot_w[:, None]
    if not np.allclose(kw, 1.0):
        coskT = coskT * kw[:, None]
        rot_w = np.concatenate([kw[DG // 2:], kw[: DG // 2]])
        sinkT_adj = sinkT_adj * rot_w[:, None]

    # per-head transposes
    q3 = np.ascontiguousarray(q.reshape(T, KH, G * D).transpose(1, 2, 0))  # (KH, 512, T)
    k3 = np.ascontiguousarray(k.transpose(1, 2, 0))                        # (KH, 128, T)

    wk_adj = np.asarray(wk, np.float32).copy()        # (KH, 2D, DG)
    wk_adj[:, D:, :] /= float(BLOCK)                  # fold 1/BLOCK of blockavg

    in_maps = []
    for h in range(N_CORES):
        in_maps.append({
            "qT": q3[h],
            "kT": k3[h],
            "wq": np.ascontiguousarray(np.asarray(wq, np.float32)[h].reshape(G * D, DG)),
            "wk": np.ascontiguousarray(wk_adj[h]),
            "cosq": cosqT, "sinq": sinqT_adj,
            "cosk": coskT, "sink": sinkT_adj,
            "maskadd": mask_add,
        })
    return in_maps


def _gather(results, lens, maxs, bsz):
    lens = [int(x) for x in lens]
    cu = np.concatenate([[0], np.cumsum(lens)]).astype(int)
    nbmax = maxs // BLOCK
    out = np.full((bsz, KH, maxs, nbmax), -1e20, dtype=np.float32)
    for h in range(N_CORES):
        oT = results[h]["outT"]                        # (BLOCK, ttot)
        for b, Lb in enumerate(lens):
            nbv = Lb // BLOCK
            out[b, h, :Lb, :nbv] = oT[:nbv, cu[b] : cu[b] + Lb].T
    return out


def _is_fast_path(cu, unpad, lens, maxs, cos_q, sin_q, cos_k, sin_k,
                  attention_mask):
    if any(L <= 0 or L % BLOCK != 0 or L > maxs for L in lens):
        return False
    if maxs % BLOCK != 0:
        return False
    canon = np.concatenate(
        [b * maxs + np.arange(L) for b, L in enumerate(lens)]
    )
    if unpad.shape != canon.shape or not np.array_equal(unpad, canon):
        return False
    for t in (cos_q, sin_q, cos_k, sin_k):
        t = np.asarray(t)
        if not all(np.array_equal(t[0], t[i]) for i in range(1, t.shape[0])):
            return False
    # the device applies the canonical block-causal+validity mask; verify the
    # given mask matches it exactly
    nb = maxs // BLOCK
    qpos = np.arange(maxs)
    bidx = np.arange(nb)
    lens_arr = np.asarray(lens)
    causal = (qpos[:, None] // BLOCK) >= bidx[None, :]
    qvalid = qpos[None, :] < lens_arr[:, None]
    bvalid = (bidx * BLOCK)[None, :] < lens_arr[:, None]
    expect = causal[None] & qvalid[:, :, None] & bvalid[:, None, :]
    am = np.asarray(attention_mask)
    if am.shape != (len(lens), 1, maxs, nb):
        return False
    return np.array_equal(am[:, 0], expect)


_CFG = KCfg()  # active configuration


def kernel(q, k, wq, wk, qnorm_w, knorm_w, cos_q, sin_q, cos_k, sin_k,
           attention_mask, cu_seqlens, unpad_indices, max_seqlen, cfg=None):
    cfg = cfg or _CFG
    q = np.asarray(q, np.float32)
    k = np.asarray(k, np.float32)
    cu = np.asarray(cu_seqlens).astype(np.int64)
    unpad = np.asarray(unpad_indices).astype(np.int64)
    maxs = int(np.asarray(max_seqlen))
    bsz = int(cu.shape[0] - 1)
    lens = [int(cu[i + 1] - cu[i]) for i in range(bsz)]

    if not _is_fast_path(cu, unpad, lens, maxs, cos_q, sin_q, cos_k, sin_k,
                         attention_mask):
        return _host_reference(q, k, wq, wk, qnorm_w, knorm_w, cos_q, sin_q,
                               cos_k, sin_k, attention_mask, cu, unpad, maxs)

    nc = _get_program(lens, cfg)
    in_maps = _host_prep(q, k, wq, wk, qnorm_w, knorm_w,
                         cos_q, sin_q, cos_k, sin_k, lens, maxs)
    res = run_bass_kernel_spmd(nc, in_maps, core_ids=list(range(N_CORES)))
    return _gather(res.results, lens, maxs, bsz)


def _host_reference(q, k, wq, wk, qnorm_w, knorm_w, cos_q, sin_q, cos_k,
                    sin_k, attention_mask, cu, unpad, maxs):
    """Pure-numpy replica of the reference (correctness fallback only)."""
    T = q.shape[0]
    bsz = int(cu.shape[0] - 1)
    nb = maxs // BLOCK
    lens = (cu[1:] - cu[:-1]).astype(int)

    def rmsnorm(x, w):
        ms = np.mean(x.astype(np.float64) ** 2, axis=-1, keepdims=True)
        return (x / np.sqrt(ms + EPS) * w).astype(np.float32)

    def rot_half(x):
        h = x.shape[-1] // 2
        return np.concatenate([-x[..., h:], x[..., :h]], axis=-1)

    qp = np.einsum("skgi,kgio->sko", q.reshape(T, KH, G, D), wq)
    qpad = np.zeros((bsz * maxs, KH, DG), np.float32)
    qpad[unpad] = qp
    qpad = qpad.reshape(bsz, maxs, KH, DG).transpose(0, 2, 1, 3)
    qpad = rmsnorm(qpad, qnorm_w)
    qpad = qpad * np.asarray(cos_q)[:, None] + rot_half(qpad) * np.asarray(sin_q)[:, None]

    kpad = np.zeros((bsz * maxs, KH, D), np.float32)
    kpad[unpad] = k
    kpad = kpad.reshape(bsz, nb, BLOCK, KH, D)
    pos = np.arange(maxs).reshape(nb, BLOCK)
    valid = pos[None] < lens[:, None, None]
    v = valid[..., None, None]
    cntv = valid.sum(-1)[..., None, None]
    kmax = np.where(v, kpad, -np.inf).max(axis=2)
    kmax = np.where(cntv > 0, kmax, 0.0)
    kavg = np.where(v, kpad, 0.0).sum(axis=2) / np.maximum(cntv, 1)
    kcat = np.concatenate([kmax, kavg], axis=-1).astype(np.float32)
    kk = np.einsum("bhsi,hio->bhso", kcat.transpose(0, 2, 1, 3), np.asarray(wk))
    kk = rmsnorm(kk, knorm_w)
    kk = kk * np.asarray(cos_k)[:, None] + rot_half(kk) * np.asarray(sin_k)[:, None]

    attn = np.einsum("bhsd,bhtd->bhst", qpad, kk) * (1.0 / np.sqrt(DG))
    return np.where(np.asarray(attention_mask), attn, -1e20).astype(np.float32)


# revision 21
# speedup vs baseline: 114.0780x; 114.0780x over previous
"""Trainium2 Bass kernel for nn_AttnGate (per-k-head gated attention scores).

Computes, for each k-head h:
  qp   = einsum('sgi,gio->so', q[:, 4h:4h+4, :], wq[h])        (T, DG)
  qn   = rope(rmsnorm(qp)) per padded position                 (per batch)
  kk   = rope(rmsnorm([blockmax(k); blockavg(k)] @ wk[h]))     (nb, DG)
  out  = where(mask, qn @ kk.T / sqrt(DG), -1e20)              (S, nb)

Sharding: one k-head per NeuronCore (8 heads, 8 cores). All device work in
transposed layout (feature on partitions, sequence on free dim):
  - Q projection: psum(DG, s) = wq_chunk(K=128, DG).T @ qT_chunk(K=128, s);
    a second projection with host-rotated weights (wqr) yields rot_half(P)
    directly, so RoPE is t1 = P*cos (DVE), t2 = Pr*sin (DVE),
    w = t1 + t2 (GPSIMD), wn = w*B (DVE).
  - RMSNorm across the partition dim: sq = Square(P) (ACT), sumsq via a
    ones-column matmul, rsq = Exp(-0.5*Ln(sumsq + 128*eps)) (ACT, both
    functions plus Square/Copy live in one pre-loaded activation table),
    then a K=1 broadcast matmul B = row.T @ rsq.  The q-side row folds
    1/sqrt(DG); the k-side row folds sqrt(128).
  - Scores: psum(nbv, s) = zk(K=DG, nbv).T @ wn(K=DG, s), then the causal
    mask is accumulated into the same psum as a rank-8 fp32 matmul
    (staircase x block-indicator, exactly -1e20 on masked entries), and
    ACT copies psum -> sbuf for the store.
  - K path: block max/sum by DVE reduction over (128, nb, 64) views; the
    1/64 of the block average is folded into the host-prescaled wk.

The host does only sharding glue: per-head transposes, table prep, and the
final gather/transpose plus -1e20 fill of padded rows/blocks.  fp32r
matmuls keep the relative error at ~4e-4 (all-fp32 config available via
KCfg for reference-exactness at ~1.5x the runtime).
"""

import math
from dataclasses import dataclass

import ml_dtypes
import numpy as np

import concourse.bacc as bacc
import concourse.bass as bass
import concourse.mybir as mybir
from concourse import tile
from concourse.bass_utils import run_bass_kernel_spmd

BLOCK = 64
KH = 8
G = 4
D = 128
DG = 128
EPS = 1e-6
N_CORES = 8
STILE = 512  # s-tile width (free dim) for the q pipeline
GRP = 4      # s-tiles per DMA group (q loads / output stores)

F32 = mybir.dt.float32
F32R = mybir.dt.float32r
BF16 = mybir.dt.bfloat16


@dataclass(frozen=True)
class KCfg:
    mm_dt: str = "f32r"   # q-projection matmul dtype: f32 | f32r | bf16
    qk_dt: str = "f32r"   # QK score matmul dtype:     f32 | f32r | bf16
    aux_dt: str = "f32r"  # sumsq/broadcast matmuls:   f32 | f32r
    rope_bf16: bool = False  # rope stage in bf16 (ACT converts psum->bf16)
    loop_n: int = 1       # repeat body in a hardware loop (timing only)
    psum_bufs: tuple = (2, 1, 2, 1, 1)  # (P, Pr, B, SS, SC)


def _mm_ap(ap, dt_name):
    if dt_name == "f32":
        return ap
    if dt_name == "f32r":
        return ap.bitcast(F32R)
    raise ValueError(dt_name)


def build_program(lens, cfg: KCfg = KCfg()):
    """Build + compile the per-core (single-head) SPMD program.

    lens: per-batch valid lengths, each divisible by BLOCK. The q/k inputs
    are packed (no padding): batch b occupies columns [cu[b], cu[b+1]).
    """
    lens = [int(x) for x in lens]
    assert all(L % BLOCK == 0 and L > 0 for L in lens)
    cu = np.concatenate([[0], np.cumsum(lens)]).astype(int)
    ttot = int(cu[-1])
    maxs = max(lens)
    nbmax = maxs // BLOCK

    nc = bacc.Bacc(None, target_bir_lowering=False, debug=False)

    assert not cfg.rope_bf16, "rope_bf16 path disabled in this revision"
    rope_dt = BF16 if cfg.rope_bf16 else F32
    # the QK matmul consumes the rope output, so its dtype follows rope_bf16
    qk_name = "bf16" if cfg.rope_bf16 else (
        "f32r" if cfg.qk_dt == "bf16" else cfg.qk_dt)
    # walrus requires fp32r matmul operands to be *produced* as fp32r:
    # declare the dram tensors / sbuf tiles with the fp32r dtype directly
    # (fp32 bits pass through DMA unchanged; DVE/ACT writes round).
    qio_dt = {"f32": F32, "f32r": F32R, "bf16": BF16}[cfg.mm_dt]
    aux_dt = F32R if cfg.aux_dt == "f32r" else F32
    qkv_dt = {"bf16": BF16, "f32r": F32R, "f32": F32}[qk_name]
    nq = G * D // 128  # 4 contraction chunks for the q projection

    qT = nc.dram_tensor("qT", [G * D, ttot], qio_dt, kind="ExternalInput").ap()
    kT = nc.dram_tensor("kT", [D, ttot], F32, kind="ExternalInput").ap()
    wq = nc.dram_tensor("wq", [G * D, DG], qio_dt, kind="ExternalInput").ap()
    wqr = nc.dram_tensor("wqr", [G * D, DG], qio_dt, kind="ExternalInput").ap()
    wk = nc.dram_tensor("wk", [2 * D, DG], aux_dt, kind="ExternalInput").ap()
    cosq = nc.dram_tensor("cosq", [DG, maxs], F32, kind="ExternalInput").ap()
    sinq = nc.dram_tensor("sinq", [DG, maxs], F32, kind="ExternalInput").ap()
    cosk = nc.dram_tensor("cosk", [DG, nbmax], F32, kind="ExternalInput").ap()
    sink = nc.dram_tensor("sink", [DG, nbmax], F32, kind="ExternalInput").ap()
    gmask = nc.dram_tensor("gmask", [8, 128], F32, kind="ExternalInput").ap()
    gblk = nc.dram_tensor("gblk", [8, STILE], F32, kind="ExternalInput").ap()
    # scalar const rows (fp32r-producible only via DMA or engine writes)
    cones = nc.dram_tensor("cones", [128, 1], aux_dt, kind="ExternalInput").ap()
    crows = nc.dram_tensor("crows", [2, 128], aux_dt, kind="ExternalInput").ap()
    cepsb = nc.dram_tensor("cepsb", [1, 1], F32, kind="ExternalInput").ap()
    outT = nc.dram_tensor("outT", [BLOCK, ttot], F32, kind="ExternalOutput").ap()


    with tile.TileContext(nc) as tc:
        with (
            tc.tile_pool(name="consts", bufs=1) as cpool,
            tc.tile_pool(name="kin", bufs=2) as kpool,
            tc.tile_pool(name="kside", bufs=2) as zpool,
            tc.tile_pool(name="qin", bufs=2) as qpool,
            tc.tile_pool(name="rope", bufs=2) as rpool,
            tc.tile_pool(name="outp", bufs=2) as opool,
            tc.tile_pool(name="pP", bufs=cfg.psum_bufs[0], space="PSUM") as pP,
            tc.tile_pool(name="pPr", bufs=cfg.psum_bufs[1], space="PSUM") as pPr,
            tc.tile_pool(name="pB", bufs=cfg.psum_bufs[2], space="PSUM") as pB,
            tc.tile_pool(name="pS", bufs=cfg.psum_bufs[3], space="PSUM") as pS,
            tc.tile_pool(name="pSC", bufs=cfg.psum_bufs[4], space="PSUM") as pSC,
        ):
            # ---- resident constants ----
            wq_sb = cpool.tile([128, nq, 128], qio_dt, name="wq_sb")
            nc.sync.dma_start(wq_sb[:], wq.rearrange("(c p) d -> p c d", p=128))
            wqr_sb = cpool.tile([128, nq, 128], qio_dt, name="wqr_sb")
            nc.sync.dma_start(wqr_sb[:], wqr.rearrange("(c p) d -> p c d", p=128))
            wk_sb = cpool.tile([128, 2, 128], aux_dt, name="wk_sb")
            nc.sync.dma_start(wk_sb[:], wk.rearrange("(c p) d -> p c d", p=128))
            cosq_sb = cpool.tile([DG, maxs], rope_dt, name="cosq_sb")
            sinq_sb = cpool.tile([DG, maxs], rope_dt, name="sinq_sb")
            cosk_sb = cpool.tile([DG, nbmax], F32, name="cosk_sb")
            sink_sb = cpool.tile([DG, nbmax], F32, name="sink_sb")
            # cosq_sb/sinq_sb are loaded chunkwise inside the batch-0 loop
            nc.sync.dma_start(cosk_sb[:], cosk[:])
            nc.sync.dma_start(sink_sb[:], sink[:])
            # additive causal mask as a rank-8 fp32 matmul into the scores
            # psum: staircase(t,s) = sum_k gmask[k, 64-8j+t] * gblk[k, s]
            gmask_sb = cpool.tile([8, 128], F32, name="gmask_sb")
            nc.sync.dma_start(gmask_sb[:], gmask[:])
            gblk_sb = cpool.tile([8, STILE], F32, name="gblk_sb")
            nc.sync.dma_start(gblk_sb[:], gblk[:])

            # rsq = Exp(-0.5*Ln(ss + 128eps)) = (ss + 128eps)^-1/2; the
            # q-side bcast row folds 1/sqrt(DG), the k-side row sqrt(128).
            ones_col = cpool.tile([128, 1], aux_dt, name="ones_col")
            nc.sync.dma_start(ones_col[:], cones[:])
            qrow = cpool.tile([1, 128], aux_dt, name="qrow")
            nc.sync.dma_start(qrow[:], crows[0:1, :])
            krow = cpool.tile([1, 128], aux_dt, name="krow")
            nc.sync.dma_start(krow[:], crows[1:2, :])
            epsb = cpool.tile([1, 1], F32, name="epsb")
            nc.sync.dma_start(epsb[:], cepsb[:])

            # Pre-load the one activation table containing Square+Ln+Exp+Copy
            # (natural_log_exp_and_others); without this the greedy table
            # chooser alternates tables around every Ln/Exp pair (~1.3us per
            # reload, 2 per s-tile).
            from concourse.hw_specs import get_activation_tables
            _tables = list(get_activation_tables(nc.m.arch).keys())
            _tid = _tables.index("natural_log_exp_and_others")
            nc.scalar.add_instruction(mybir.InstLoadActFuncSet(
                name=nc.get_next_instruction_name(), act_func_set_id=_tid,
                ins=[], outs=[]))

            def body():
                for b, Lb in enumerate(lens):
                    nbv = Lb // BLOCK
                    c0 = int(cu[b])
                    # ---------- K path ----------
                    kt = kpool.tile([128, maxs], F32, name="kt", tag="kt")
                    nc.sync.dma_start(kt[:, :Lb], kT[:, c0 : c0 + Lb])
                    kview = kt[:, :Lb].rearrange("p (n c) -> p n c", c=BLOCK)
                    kmax = zpool.tile([128, nbmax], aux_dt, name="kmax", tag="kmax")
                    nc.vector.reduce_max(kmax[:, :nbv], kview, axis=mybir.AxisListType.X)
                    ksum = zpool.tile([128, nbmax], aux_dt, name="ksum", tag="ksum")
                    with nc.allow_low_precision("fp32r blocksum; accum is fp32"):
                        nc.vector.reduce_sum(ksum[:, :nbv], kview, axis=mybir.AxisListType.X)
                    kkp = pP.tile([128, STILE], F32, name="kkp", tag="P")[:, :nbv]
                    # wk low half is pre-scaled by 1/BLOCK on the host
                    nc.tensor.matmul(kkp, wk_sb[:, 0, :], kmax[:, :nbv],
                                     start=True, stop=False)
                    nc.tensor.matmul(kkp, wk_sb[:, 1, :], ksum[:, :nbv],
                                     start=False, stop=True)
                    sqk = zpool.tile([128, nbmax], aux_dt, name="sqk", tag="sqk")
                    nc.scalar.square(sqk[:, :nbv], kkp)
                    ssk = pS.tile([1, STILE], F32, name="ssk", tag="SS")[:, :nbv]
                    nc.tensor.matmul(ssk, ones_col[:], sqk[:, :nbv],
                                     start=True, stop=True)
                    lnk = zpool.tile([1, nbmax], F32, name="lnk", tag="lnk")
                    nc.scalar.activation(lnk[:, :nbv], ssk,
                                         mybir.ActivationFunctionType.Ln,
                                         bias=epsb[:, :])
                    rsqk = zpool.tile([1, nbmax], aux_dt, name="rsqk", tag="rsqk")
                    nc.scalar.activation(rsqk[:, :nbv], lnk[:, :nbv],
                                         mybir.ActivationFunctionType.Exp,
                                         scale=-0.5)
                    bk = pB.tile([128, STILE], F32, name="bk", tag="B")[:, :nbv]
                    nc.tensor.matmul(bk, krow[:], rsqk[:, :nbv],
                                     start=True, stop=True)
                    # rope(kk) * bk -> zk
                    t1k = zpool.tile([128, nbmax], F32, name="t1k", tag="t1k")
                    nc.vector.tensor_mul(t1k[:, :nbv], kkp, cosk_sb[:, :nbv])
                    t2k = zpool.tile([128, nbmax], F32, name="t2k", tag="t2k")
                    nc.vector.tensor_mul(t2k[:64, :nbv], kkp[64:128, :], sink_sb[:64, :nbv])
                    nc.vector.tensor_mul(t2k[64:128, :nbv], kkp[:64, :], sink_sb[64:128, :nbv])
                    wkk = zpool.tile([128, nbmax], F32, name="wkk", tag="wkk")
                    nc.gpsimd.tensor_add(wkk[:, :nbv], t1k[:, :nbv], t2k[:, :nbv])
                    zk = zpool.tile([128, nbmax], qkv_dt, name="zk", tag="zk")
                    nc.vector.tensor_mul(zk[:, :nbv], wkk[:, :nbv], bk)

                    # ---------- Q path ----------
                    # q loads and output stores are batched in groups of
                    # GRP s-tiles to amortize per-DMA dispatch overhead.
                    qT3 = qT.rearrange("(c p) t -> p c t", p=128)
                    n_tiles = (Lb + STILE - 1) // STILE
                    for g0 in range(0, n_tiles, GRP):
                        gtiles = min(GRP, n_tiles - g0)
                        ga = g0 * STILE
                        gw = min(GRP * STILE, Lb - ga)
                        qt = qpool.tile([128, nq, GRP * STILE], qio_dt,
                                        name="qt", tag="qt")
                        nc.sync.dma_start(qt[:, :, :gw],
                                          qT3[:, :, c0 + ga : c0 + ga + gw])
                        if b == 0:
                            # JIT-load the resident rope tables chunkwise so
                            # compute starts before the whole 4MB lands
                            nc.sync.dma_start(cosq_sb[:, ga : ga + gw],
                                              cosq[:, ga : ga + gw])
                            nc.sync.dma_start(sinq_sb[:, ga : ga + gw],
                                              sinq[:, ga : ga + gw])
                        osb = opool.tile([BLOCK, GRP * STILE], F32,
                                         name="osb", tag="osb")
                        for jj in range(gtiles):
                            j = g0 + jj
                            s0 = j * STILE
                            o0 = jj * STILE
                            w = min(STILE, Lb - s0)
                            P = pP.tile([128, STILE], F32, name="P", tag="P")[:, :w]
                            for c in range(nq):
                                nc.tensor.matmul(P, wq_sb[:, c, :],
                                                 qt[:, c, o0 : o0 + w],
                                                 start=(c == 0), stop=(c == nq - 1))
                            Pr = pPr.tile([128, STILE], F32, name="Pr", tag="Pr")[:, :w]
                            for c in range(nq):
                                nc.tensor.matmul(Pr, wqr_sb[:, c, :],
                                                 qt[:, c, o0 : o0 + w],
                                                 start=(c == 0), stop=(c == nq - 1))
                            sq = rpool.tile([128, STILE], aux_dt, name="sq", tag="sq")
                            nc.scalar.square(sq[:, :w], P)
                            ss = pS.tile([1, STILE], F32, name="ss", tag="SS")[:, :w]
                            nc.tensor.matmul(ss, ones_col[:], sq[:, :w],
                                             start=True, stop=True)
                            lnq = rpool.tile([1, STILE], F32, name="lnq", tag="lnq")
                            nc.scalar.activation(lnq[:, :w], ss,
                                                 mybir.ActivationFunctionType.Ln,
                                                 bias=epsb[:, :])
                            rsq = rpool.tile([1, STILE], aux_dt, name="rsq", tag="rsq")
                            nc.scalar.activation(rsq[:, :w], lnq[:, :w],
                                                 mybir.ActivationFunctionType.Exp,
                                                 scale=-0.5)
                            B = pB.tile([128, STILE], F32, name="B", tag="B")[:, :w]
                            nc.tensor.matmul(B, qrow[:], rsq[:, :w],
                                             start=True, stop=True)
                            t1 = rpool.tile([128, STILE], rope_dt, name="t1", tag="t1")
                            nc.vector.tensor_mul(t1[:, :w], P, cosq_sb[:, s0 : s0 + w])
                            t2 = rpool.tile([128, STILE], rope_dt, name="t2", tag="t2")
                            nc.vector.tensor_mul(t2[:, :w], Pr, sinq_sb[:, s0 : s0 + w])
                            wr = rpool.tile([128, STILE], rope_dt, name="wr", tag="wr")
                            nc.gpsimd.tensor_add(wr[:, :w], t1[:, :w], t2[:, :w])
                            wn = rpool.tile([128, STILE], qkv_dt, name="wn", tag="wn")
                            nc.vector.tensor_mul(wn[:, :w], wr[:, :w], B)
                            S = pSC.tile([BLOCK, STILE], F32, name="S", tag="SC")[:nbv, :w]
                            nc.tensor.matmul(S, zk[:, :nbv], wn[:, :w],
                                             start=True, stop=False)
                            nc.tensor.matmul(S, gmask_sb[:, 64 - 8 * j : 64 - 8 * j + nbv],
                                             gblk_sb[:, :w], start=False, stop=True)
                            nc.scalar.copy(osb[:nbv, o0 : o0 + w], S)
                        nc.sync.dma_start(outT[0:nbv, c0 + ga : c0 + ga + gw],
                                          osb[:nbv, :gw])

            if cfg.loop_n > 1:
                with tc.For_i(0, cfg.loop_n, 1):
                    body()
            else:
                body()

    nc.compile()
    return nc


# ---------------------------------------------------------------------------
# Host-side orchestration
# ---------------------------------------------------------------------------

_PROG_CACHE: dict = {}


def _get_program(lens, cfg: KCfg):
    key = (tuple(lens), cfg)
    if key not in _PROG_CACHE:
        _PROG_CACHE[key] = build_program(lens, cfg)
    return _PROG_CACHE[key]


def _host_prep(q, k, wq, wk, qnorm_w, knorm_w, cos_q, sin_q, cos_k, sin_k,
               lens, maxs, cfg=None):
    """Build the 8 per-core input dicts (fast path, packed layout)."""
    cfg = cfg or _CFG
    q_np = ml_dtypes.bfloat16 if cfg.mm_dt == "bf16" else np.float32
    T = q.shape[0]
    nbmax = maxs // BLOCK
    lens = [int(x) for x in lens]

    # rope tables (identical across batches on the fast path).  The q-side
    # rotation signs live in the rotated projection weights (wqr), so sinq
    # stays plain; the k-side uses the split-half scheme with pre-negated
    # low half.
    cosqT = np.ascontiguousarray(cos_q[0].T)          # (DG, maxs)
    sinqT = np.ascontiguousarray(sin_q[0].T)
    coskT = np.ascontiguousarray(cos_k[0].T)          # (DG, nbmax)
    sinkT_adj = np.ascontiguousarray(sin_k[0].T).copy()
    sinkT_adj[: DG // 2] *= -1.0

    # rank-8 additive causal mask factors: staircase (t,s) per s-tile j is
    # gmask[:, 64-8j+t].T @ gblk[:, s_rel]
    cidx = np.arange(128)
    kidx = np.arange(8)
    gmask = np.where(cidx[None, :] > 64 + kidx[:, None], -1e20, 0.0).astype(np.float32)
    sidx = np.arange(STILE)
    gblk = (sidx[None, :] // BLOCK == kidx[:, None]).astype(np.float32)

    # rmsnorm weights are folded in only if they are all-ones (reference
    # always uses ones); otherwise fold into wq/wk? they multiply post-norm,
    # positionwise over DG — fold into cos/sin tables instead.
    qw = np.asarray(qnorm_w, np.float32)
    kw = np.asarray(knorm_w, np.float32)
    if not np.allclose(qw, 1.0):
        cosqT = cosqT * qw[:, None]
        rot_w = np.concatenate([qw[DG // 2:], qw[: DG // 2]])
        # rope mixes dims: y*w -> (y*w)*cos + rot(y*w)*sin. w applies to y
        # BEFORE rope per reference.  rot(y*w)[d] uses w[rot(d)].
        sinqT = sinqT * rot_w[:, None]
    if not np.allclose(kw, 1.0):
        coskT = coskT * kw[:, None]
        rot_w = np.concatenate([kw[DG // 2:], kw[: DG // 2]])
        sinkT_adj = sinkT_adj * rot_w[:, None]

    # per-head transposes
    q3 = np.ascontiguousarray(q.reshape(T, KH, G * D).transpose(1, 2, 0))  # (KH, 512, T)
    if q_np != np.float32:
        q3 = q3.astype(q_np)
    k3 = np.ascontiguousarray(k.transpose(1, 2, 0))                        # (KH, 128, T)

    wk_adj = np.asarray(wk, np.float32).copy()        # (KH, 2D, DG)
    wk_adj[:, D:, :] /= float(BLOCK)                  # fold 1/BLOCK of blockavg

    in_maps = []
    for h in range(N_CORES):
        wq_h = np.ascontiguousarray(np.asarray(wq, np.float32)[h].reshape(G * D, DG))
        wqr_h = np.empty_like(wq_h)
        wqr_h[:, : DG // 2] = -wq_h[:, DG // 2 :]
        wqr_h[:, DG // 2 :] = wq_h[:, : DG // 2]
        if q_np != np.float32:
            wq_h = wq_h.astype(q_np)
            wqr_h = wqr_h.astype(q_np)
        in_maps.append({
            "qT": q3[h],
            "kT": k3[h],
            "wq": wq_h,
            "wqr": np.ascontiguousarray(wqr_h),
            "wk": np.ascontiguousarray(wk_adj[h]),
            "cosq": cosqT, "sinq": sinqT,
            "cosk": coskT, "sink": sinkT_adj,
            "gmask": gmask, "gblk": gblk,
            "cones": np.ones((128, 1), np.float32),
            "crows": np.stack([np.full(128, 1.0, np.float32),
                               np.full(128, math.sqrt(128.0), np.float32)]),
            "cepsb": np.full((1, 1), 128.0 * EPS, np.float32),
        })
    return in_maps


def _gather(results, lens, maxs, bsz):
    lens = [int(x) for x in lens]
    cu = np.concatenate([[0], np.cumsum(lens)]).astype(int)
    nbmax = maxs // BLOCK
    out = np.full((bsz, KH, maxs, nbmax), -1e20, dtype=np.float32)
    for h in range(N_CORES):
        oT = results[h]["outT"]                        # (BLOCK, ttot)
        for b, Lb in enumerate(lens):
            nbv = Lb // BLOCK
            out[b, h, :Lb, :nbv] = oT[:nbv, cu[b] : cu[b] + Lb].T
    return out


def _is_fast_path(cu, unpad, lens, maxs, cos_q, sin_q, cos_k, sin_k,
                  attention_mask):
    if any(L <= 0 or L % BLOCK != 0 or L > maxs for L in lens):
        return False
    if maxs % BLOCK != 0:
        return False
    canon = np.concatenate(
        [b * maxs + np.arange(L) for b, L in enumerate(lens)]
    )
    if unpad.shape != canon.shape or not np.array_equal(unpad, canon):
        return False
    for t in (cos_q, sin_q, cos_k, sin_k):
        t = np.asarray(t)
        if not all(np.array_equal(t[0], t[i]) for i in range(1, t.shape[0])):
            return False
    # the device applies the canonical block-causal+validity mask; verify the
    # given mask matches it exactly
    nb = maxs // BLOCK
    qpos = np.arange(maxs)
    bidx = np.arange(nb)
    lens_arr = np.asarray(lens)
    causal = (qpos[:, None] // BLOCK) >= bidx[None, :]
    qvalid = qpos[None, :] < lens_arr[:, None]
    bvalid = (bidx * BLOCK)[None, :] < lens_arr[:, None]
    expect = causal[None] & qvalid[:, :, None] & bvalid[:, None, :]
    am = np.asarray(attention_mask)
    if am.shape != (len(lens), 1, maxs, nb):
        return False
    return np.array_equal(am[:, 0], expect)


_CFG = KCfg()  # active configuration


def kernel(q, k, wq, wk, qnorm_w, knorm_w, cos_q, sin_q, cos_k, sin_k,
           attention_mask, cu_seqlens, unpad_indices, max_seqlen, cfg=None):
    cfg = cfg or _CFG
    q = np.asarray(q, np.float32)
    k = np.asarray(k, np.float32)
    cu = np.asarray(cu_seqlens).astype(np.int64)
    unpad = np.asarray(unpad_indices).astype(np.int64)
    maxs = int(np.asarray(max_seqlen))
    bsz = int(cu.shape[0] - 1)
    lens = [int(cu[i + 1] - cu[i]) for i in range(bsz)]

    if not _is_fast_path(cu, unpad, lens, maxs, cos_q, sin_q, cos_k, sin_k,
                         attention_mask):
        return _host_reference(q, k, wq, wk, qnorm_w, knorm_w, cos_q, sin_q,
                               cos_k, sin_k, attention_mask, cu, unpad, maxs)

    nc = _get_program(lens, cfg)
    in_maps = _host_prep(q, k, wq, wk, qnorm_w, knorm_w,
                         cos_q, sin_q, cos_k, sin_k, lens, maxs, cfg=cfg)
    res = run_bass_kernel_spmd(nc, in_maps, core_ids=list(range(N_CORES)))
    return _gather(res.results, lens, maxs, bsz)


def _host_reference(q, k, wq, wk, qnorm_w, knorm_w, cos_q, sin_q, cos_k,
                    sin_k, attention_mask, cu, unpad, maxs):
    """Pure-numpy replica of the reference (correctness fallback only)."""
    T = q.shape[0]
    bsz = int(cu.shape[0] - 1)
    nb = maxs // BLOCK
    lens = (cu[1:] - cu[:-1]).astype(int)

    def rmsnorm(x, w):
        ms = np.mean(x.astype(np.float64) ** 2, axis=-1, keepdims=True)
        return (x / np.sqrt(ms + EPS) * w).astype(np.float32)

    def rot_half(x):
        h = x.shape[-1] // 2
        return np.concatenate([-x[..., h:], x[..., :h]], axis=-1)

    qp = np.einsum("skgi,kgio->sko", q.reshape(T, KH, G, D), wq)
    qpad = np.zeros((bsz * maxs, KH, DG), np.float32)
    qpad[unpad] = qp
    qpad = qpad.reshape(bsz, maxs, KH, DG).transpose(0, 2, 1, 3)
    qpad = rmsnorm(qpad, qnorm_w)
    qpad = qpad * np.asarray(cos_q)[:, None] + rot_half(qpad) * np.asarray(sin_q)[:, None]

    kpad = np.zeros((bsz * maxs, KH, D), np.float32)
    kpad[unpad] = k
    kpad = kpad.reshape(bsz, nb, BLOCK, KH, D)
    pos = np.arange(maxs).reshape(nb, BLOCK)
    valid = pos[None] < lens[:, None, None]
    v = valid[..., None, None]
    cntv = valid.sum(-1)[..., None, None]
    kmax = np.where(v, kpad, -np.inf).max(axis=2)
    kmax = np.where(cntv > 0, kmax, 0.0)
    kavg = np.where(v, kpad, 0.0).sum(axis=2) / np.maximum(cntv, 1)
    kcat = np.concatenate([kmax, kavg], axis=-1).astype(np.float32)
    kk = np.einsum("bhsi,hio->bhso", kcat.transpose(0, 2, 1, 3), np.asarray(wk))
    kk = rmsnorm(kk, knorm_w)
    kk = kk * np.asarray(cos_k)[:, None] + rot_half(kk) * np.asarray(sin_k)[:, None]

    attn = np.einsum("bhsd,bhtd->bhst", qpad, kk) * (1.0 / np.sqrt(DG))
    return np.where(np.asarray(attention_mask), attn, -1e20).astype(np.float32)
